# revision 4
# baseline (speedup 1.0000x reference)
"""Camera back-projection (truncated depth field) Trainium2 kernel, v2.

out[b,0,i,j,k] = relu(1 - 128*|depth[b,0,vi(j,k),ui(i,k)] - zc_k|) with
frustum/validity masking, where (u,v) are pinhole projections of the voxel
grid and u == v index maps (square image, symmetric grid). 8 cores, 2
batches/core, pure data parallel.

Device pipeline (per batch, per 8-k chunk; KCH=8, 16 chunks/batch):
  gather (GPSIMD ap_gather): A{rt}[r, (k,i), 2] = W2{rt}[r, ui(i,k), :]
      W2 holds (hi,lo) fp16 pairs of the CENTERED window W' = depth - cam_dist
      (|W'| <= 0.5 so fp16 rounding err <= 2^-13; poison +100 for invalid).
      Column gather is exact; indices shared across partitions.
  P-gen (DVE): P{rt}[p, (k,j)] = (vi_rep[(k,j)] == p + 128*rt)  fp16 one-hot
      via tensor_scalar is_equal against a per-partition iota column.
  stage B (PE): psB[j, (k,i)] = z_k (aug matmul, first, start=True)
      + sum_r P{rt}[r,(k,j)] * A{rt}[r,(k,i),0]   (hi parts, stride-2 rhs)
      = W'[vi,ui] - zc' exactly (zc' = -z_k exact fp16; invalid -> ~100).
  tent (ACT): out = relu(1 - 128*|psB|) in f32, straight to output DMA.
Host: out[b,0,i,j,k] = outdev[b][j,k,i] (pure transpose).
Max error bound: 128 * 2^-13 = 0.0156 (fp16 rounding of W' only).
"""
import sys
import numpy as np

sys.path.insert(0, "/opt/trn_rl_repo")

RES = 128
IMG = 480
N = 16
NCORES = 8
BPC = N // NCORES          # batches per core
WIN = 252                  # depth window rows/cols actually used
WPAD = 256                 # padded (poison) to 2 partition tiles
KCH = 8                    # k's per pipeline chunk
NCHUNK = RES // KCH        # 16
POISON = np.float32(100.0)

P = 128
NF = KCH * RES             # free size per chunk (1024)

_nc_cache = {}


def _build_program():
    import concourse.bacc as bacc
    import concourse.mybir as mybir
    import concourse.tile as tile

    nc = bacc.Bacc(None, target_bir_lowering=False, debug=False)
    with tile.TileContext(nc) as tc:
        with tc.tile_pool(name="dram", bufs=1, space="DRAM") as dram:
            w2s, idxs, vis, znegs, outs = {}, {}, {}, {}, {}
            pcol_d = dram.tile([P, 2], mybir.dt.float32,
                               kind="ExternalInput", uniquify=False, name="pcol")
            ones1_d = dram.tile([1, P], mybir.dt.float16,
                                kind="ExternalInput", uniquify=False, name="ones1")
            for b in range(BPC):
                w2s[b] = dram.tile([2, P, 2 * WPAD], mybir.dt.float16,
                                   kind="ExternalInput", uniquify=False, name=f"w2_{b}")
                idxs[b] = dram.tile([P, NCHUNK * NF // 16], mybir.dt.int16,
                                    kind="ExternalInput", uniquify=False, name=f"idx{b}")
                vis[b] = dram.tile([P, NCHUNK * NF], mybir.dt.float16,
                                   kind="ExternalInput", uniquify=False, name=f"vi{b}")
                znegs[b] = dram.tile([1, NCHUNK * NF], mybir.dt.float16,
                                     kind="ExternalInput", uniquify=False, name=f"zneg{b}")
                outs[b] = dram.tile([RES, RES * RES], mybir.dt.float32,
                                    kind="ExternalOutput", uniquify=False, name=f"outdev{b}")

            with (
                tc.tile_pool(name="sb", bufs=1) as sb,
                tc.tile_pool(name="ps", bufs=1, space="PSUM") as ps,
            ):
                pcol_sb = sb.tile([P, 2], mybir.dt.float32, name="pcol_sb")
                ones1_sb = sb.tile([1, P], mybir.dt.float16, name="ones1_sb")
                nc.sync.dma_start(pcol_sb[:], pcol_d[:])
                nc.sync.dma_start(ones1_sb[:], ones1_d[:])

                for b in range(BPC):
                    w2_sb = {}
                    for rt in range(2):
                        t = sb.tile([P, 2 * WPAD], mybir.dt.float16,
                                    name=f"w2_{rt}_{b}", tag=f"w2_{rt}", bufs=2)
                        nc.sync.dma_start(t[:], w2s[b][rt])
                        w2_sb[rt] = t
                    idx_sb = sb.tile([P, NCHUNK * NF // 16], mybir.dt.int16,
                                     name=f"idx_{b}", tag="idx", bufs=2)
                    nc.sync.dma_start(idx_sb[:], idxs[b][:])
                    vi_sb = sb.tile([P, NCHUNK * NF], mybir.dt.float16,
                                    name=f"vi_{b}", tag="vi", bufs=2)
                    nc.sync.dma_start(vi_sb[:], vis[b][:])
                    zneg_sb = sb.tile([1, NCHUNK * NF], mybir.dt.float16,
                                      name=f"zneg_{b}", tag="zneg", bufs=2)
                    nc.sync.dma_start(zneg_sb[:], znegs[b][:])

                    for ch in range(NCHUNK):
                        fsl = slice(ch * NF, (ch + 1) * NF)
                        isl = slice(ch * NF // 16, (ch + 1) * NF // 16)

                        A = {}
                        for rt in range(2):
                            A[rt] = sb.tile([P, NF, 2], mybir.dt.float16,
                                            name=f"A{rt}_{b}_{ch}", tag=f"A{rt}", bufs=3)
                            nc.gpsimd.ap_gather(
                                A[rt][:], w2_sb[rt][:], idx_sb[:, isl],
                                channels=P, num_elems=WPAD, d=2, num_idxs=NF,
                            )
                        Pt = {}
                        for rt in range(2):
                            Pt[rt] = sb.tile([P, NF], mybir.dt.float16,
                                             name=f"P{rt}_{b}_{ch}", tag=f"P{rt}", bufs=3)
                            nc.vector.tensor_scalar(
                                Pt[rt][:], vi_sb[:, fsl],
                                scalar1=pcol_sb[:, rt:rt + 1], scalar2=None,
                                op0=mybir.AluOpType.is_equal,
                            )

                        psB = ps.tile([P, NF], mybir.dt.float32,
                                      name=f"psB_{b}_{ch}", tag="psB", bufs=3)
                        for h in range(2):
                            hsl = slice(ch * NF + h * (NF // 2),
                                        ch * NF + (h + 1) * (NF // 2))
                            nc.tensor.matmul(psB[:, h * (NF // 2):(h + 1) * (NF // 2)],
                                             ones1_sb[:], zneg_sb[:, hsl],
                                             start=True, stop=False, skip_group_check=True)
                        for kc in range(KCH):
                            ksl = slice(kc * RES, (kc + 1) * RES)
                            for rt in range(2):
                                nc.tensor.matmul(
                                    psB[:, ksl], Pt[rt][:, ksl], A[rt][:, ksl, 0],
                                    start=False, stop=(kc == KCH - 1 and rt == 1),
                                    skip_group_check=True,
                                )

                        aa = sb.tile([P, NF], mybir.dt.float32,
                                     name=f"aa_{b}_{ch}", tag="aa", bufs=3)
                        nc.scalar.activation(aa[:], psB[:],
                                             mybir.ActivationFunctionType.Abs)
                        ob = sb.tile([P, NF], mybir.dt.float32,
                                     name=f"ob_{b}_{ch}", tag="ob", bufs=3)
                        nc.scalar.activation(ob[:], aa[:],
                                             mybir.ActivationFunctionType.Relu,
                                             bias=1.0, scale=-128.0)
                        nc.sync.dma_start(outs[b][:, fsl], ob[:])
    nc.compile()
    return nc


def _host_precompute(depth, fl, cd):
    """Per-batch device inputs. Index math in float32, matching the jax
    reference op-for-op."""
    f32 = np.float32
    res = RES
    c = ((np.arange(res, dtype=f32) + f32(0.5)) / f32(res)) - f32(0.5)
    zc = f32(cd) - c                        # [k]
    kvalid = zc > 0
    with np.errstate(divide="ignore", invalid="ignore"):
        u = (f32(fl) * c)[:, None] / zc[None, :] + f32((IMG - 1) * 0.5)  # [i,k] == [j,k]
    ui = np.clip(np.round(u), 0, IMG - 1).astype(np.int64)
    mu = (u >= 0) & (u <= IMG - 1) & kvalid[None, :]

    if mu.any():
        cmin = int(ui[mu].min())
        cmax = int(ui[mu].max())
    else:
        cmin = cmax = 0
    if (cmax - cmin) >= WIN:
        raise NotImplementedError("projection span exceeds window")
    base = min(cmin, IMG - WIN)

    w = depth[base:base + WIN, base:base + WIN].astype(f32) - f32(cd)
    w[depth[base:base + WIN, base:base + WIN] <= 0] = POISON
    wpad = np.full((2 * P, WPAD), POISON, dtype=f32)
    wpad[:WIN, :WIN] = w
    w_hi = wpad.astype(np.float16)
    w_lo = (wpad - w_hi.astype(f32)).astype(np.float16)
    # w2[rt, p, 2c:2c+2] = (hi, lo) of window row rt*128+p, col c
    w2 = np.empty((2, P, 2 * WPAD), dtype=np.float16)
    for rt in range(2):
        w2[rt, :, 0::2] = w_hi[rt * P:(rt + 1) * P]
        w2[rt, :, 1::2] = w_lo[rt * P:(rt + 1) * P]

    # windowed column indices per (k, i); invalid -> poison col 252
    uiw = np.where(mu, ui - base, WIN).astype(np.int16)        # [i, k]
    # per chunk: idx list over (k within chunk, i), wrapped in 16 partitions
    idx_flat = uiw.T.reshape(NCHUNK, KCH * res)                # [(chunk k), i]
    idx_t = np.zeros((P, NCHUNK * NF // 16), dtype=np.int16)
    for ch in range(NCHUNK):
        wrap = idx_flat[ch].reshape(NF // 16, 16).T            # [16, 64]
        for g in range(8):
            idx_t[g * 16:(g + 1) * 16, ch * NF // 16:(ch + 1) * NF // 16] = wrap

    # vi_rep [(k,j)] fp16, replicated across partitions; invalid -> 252
    vi_rep = np.tile(uiw.T.reshape(1, NCHUNK * NF).astype(np.float16), (P, 1))

    # zneg [(k,i)]: z_k = -zc' = c_k, exact in fp16; invalid k -> +poison
    zk = np.where(kvalid, c, POISON).astype(np.float16)
    zneg = np.repeat(zk, res)[None, :]                         # [1, 128*128]

    return w2, idx_t, vi_rep, zneg


def kernel(depth_t, fl, cam_dist):
    from concourse.bass_utils import run_bass_kernel_spmd

    depth_t = np.asarray(depth_t)
    fl = np.asarray(fl).reshape(N)
    cam_dist = np.asarray(cam_dist).reshape(N)

    if "nc" not in _nc_cache:
        _nc_cache["nc"] = _build_program()
    nc = _nc_cache["nc"]

    pcol = np.stack([np.arange(P, dtype=np.float32),
                     np.arange(P, dtype=np.float32) + P], axis=1)
    ones1 = np.ones((1, P), dtype=np.float16)

    in_maps = []
    for core in range(NCORES):
        m = {"pcol": pcol, "ones1": ones1}
        for b in range(BPC):
            g = core * BPC + b
            w2, idx_t, vi_rep, zneg = _host_precompute(
                depth_t[g, 0], fl[g], cam_dist[g])
            m[f"w2_{b}"] = w2
            m[f"idx{b}"] = idx_t
            m[f"vi{b}"] = vi_rep
            m[f"zneg{b}"] = zneg
        in_maps.append(m)

    globals()["_last_in_maps"] = in_maps
    r = run_bass_kernel_spmd(nc, in_maps, list(range(NCORES)))

    out = np.empty((N, 1, RES, RES, RES), dtype=np.float32)
    for core in range(NCORES):
        for b in range(BPC):
            g = core * BPC + b
            od = r.results[core][f"outdev{b}"].reshape(RES, RES, RES)  # [j,k,i]
            out[g, 0] = od.transpose(2, 0, 1)
    return out


# revision 5
# speedup vs baseline: 10.5941x; 10.5941x over previous
"""Camera back-projection (truncated depth field) Trainium2 kernel, v3.

out[b,0,i,j,k] = relu(1 - 128*|depth[b,0,vi(j,k),ui(i,k)] - zc_k|) with
frustum/validity masking; u == v index maps. 8 cores, 2 batches/core.

Per chunk (4 k's, NF=512):
  QP[ct] (DVE): one-hot (vi_rep == c+128*ct) fp16 — serves BOTH stages
    (stage A moving operand AND stage B stationary; u == v).
  stage A (PE): psA[(rt), (k,i)] = z_k (aug MM first) + sum_c winT[c,r]*QP
    = W'[r, ui(i,k)] - zc'(k) in f32 psum.  W' = depth - cam_dist centered,
    |W'| <= 0.5 -> fp16 err <= 2^-13; poison +100 invalid.
  F (ACT): Abs(128*psA) -> fp16 (scale before cast keeps err ~2e-4).
  stage B (PE): psB[j,(k,i)] = sum_rt QP[rt]^T F[rt] = F at row vi(j,k).
  out (ACT): relu(1 - psB) f32 -> DMA.
Max err ~ 128*2^-13 + 5e-4 ~ 0.016 < 0.02.
"""
import sys
import numpy as np

sys.path.insert(0, "/opt/trn_rl_repo")

RES = 128
IMG = 480
N = 16
NCORES = 8
BPC = N // NCORES
WIN = 252
WPAD = 256
KCH = 4
NCHUNK = RES // KCH        # 32
POISON = np.float32(100.0)

P = 128
NF = KCH * RES             # 512

_nc_cache = {}


def _build_program():
    import concourse.bacc as bacc
    import concourse.mybir as mybir
    import concourse.tile as tile

    nc = bacc.Bacc(None, target_bir_lowering=False, debug=False)
    with tile.TileContext(nc) as tc:
        with tc.tile_pool(name="dram", bufs=1, space="DRAM") as dram:
            wts, vis, znegs, outs = {}, {}, {}, {}
            pcol_d = dram.tile([P, 2], mybir.dt.float32,
                               kind="ExternalInput", uniquify=False, name="pcol")
            ones1_d = dram.tile([1, P], mybir.dt.float16,
                                kind="ExternalInput", uniquify=False, name="ones1")
            for b in range(BPC):
                wts[b] = dram.tile([2, P, WPAD], mybir.dt.float16,
                                   kind="ExternalInput", uniquify=False, name=f"wt{b}")
                vis[b] = dram.tile([P, NCHUNK * NF], mybir.dt.float16,
                                   kind="ExternalInput", uniquify=False, name=f"vi{b}")
                znegs[b] = dram.tile([1, NCHUNK * NF], mybir.dt.float16,
                                     kind="ExternalInput", uniquify=False, name=f"zneg{b}")
                outs[b] = dram.tile([RES, RES * RES], mybir.dt.float32,
                                    kind="ExternalOutput", uniquify=False, name=f"outdev{b}")

            with (
                tc.tile_pool(name="sb", bufs=1) as sb,
                tc.tile_pool(name="ps", bufs=1, space="PSUM") as ps,
            ):
                pcol_sb = sb.tile([P, 2], mybir.dt.float32, name="pcol_sb")
                ones1_sb = sb.tile([1, P], mybir.dt.float16, name="ones1_sb")
                nc.sync.dma_start(pcol_sb[:], pcol_d[:])
                nc.sync.dma_start(ones1_sb[:], ones1_d[:])

                for b in range(BPC):
                    wt_sb = {}
                    for ct in range(2):
                        t = sb.tile([P, WPAD], mybir.dt.float16,
                                    name=f"wt_{ct}_{b}", tag=f"wt_{ct}", bufs=2)
                        nc.sync.dma_start(t[:], wts[b][ct])
                        wt_sb[ct] = t
                    vi_sb = sb.tile([P, NCHUNK * NF], mybir.dt.float16,
                                    name=f"vi_{b}", tag="vi", bufs=2)
                    nc.sync.dma_start(vi_sb[:], vis[b][:])
                    zneg_sb = sb.tile([1, NCHUNK * NF], mybir.dt.float16,
                                      name=f"zneg_{b}", tag="zneg", bufs=2)
                    nc.sync.dma_start(zneg_sb[:], znegs[b][:])

                    for ch in range(NCHUNK):
                        fsl = slice(ch * NF, (ch + 1) * NF)

                        QP = {}
                        for ct in range(2):
                            QP[ct] = sb.tile([P, NF], mybir.dt.float16,
                                             name=f"QP{ct}_{b}_{ch}", tag=f"QP{ct}", bufs=3)
                            nc.vector.tensor_scalar(
                                QP[ct][:], vi_sb[:, fsl],
                                scalar1=pcol_sb[:, ct:ct + 1], scalar2=None,
                                op0=mybir.AluOpType.is_equal,
                            )

                        # psA [128, 2*NF]: halves are the two r-tiles
                        psA = ps.tile([P, 2 * NF], mybir.dt.float32,
                                      name=f"psA_{b}_{ch}", tag="psA", bufs=2)
                        for rt in range(2):
                            hsl = slice(rt * NF, (rt + 1) * NF)
                            nc.tensor.matmul(psA[:, hsl], ones1_sb[:],
                                             zneg_sb[:, fsl],
                                             start=True, stop=False,
                                             skip_group_check=True)
                        for rt in range(2):
                            hsl = slice(rt * NF, (rt + 1) * NF)
                            for ct in range(2):
                                nc.tensor.matmul(
                                    psA[:, hsl],
                                    wt_sb[ct][:, rt * P:(rt + 1) * P],
                                    QP[ct][:],
                                    start=False, stop=(ct == 1),
                                    skip_group_check=True,
                                )

                        F = sb.tile([P, 2 * NF], mybir.dt.float16,
                                    name=f"F_{b}_{ch}", tag="F", bufs=3)
                        nc.scalar.activation(F[:], psA[:],
                                             mybir.ActivationFunctionType.Abs,
                                             scale=128.0)

                        psB = ps.tile([P, NF], mybir.dt.float32,
                                      name=f"psB_{b}_{ch}", tag="psB", bufs=3)
                        for kc in range(KCH):
                            ksl = slice(kc * RES, (kc + 1) * RES)
                            for rt in range(2):
                                nc.tensor.matmul(
                                    psB[:, ksl], QP[rt][:, ksl],
                                    F[:, rt * NF + kc * RES:rt * NF + (kc + 1) * RES],
                                    start=(rt == 0), stop=(rt == 1),
                                )

                        ob = sb.tile([P, NF], mybir.dt.float32,
                                     name=f"ob_{b}_{ch}", tag="ob", bufs=3)
                        nc.scalar.activation(ob[:], psB[:],
                                             mybir.ActivationFunctionType.Relu,
                                             bias=1.0, scale=-1.0)
                        nc.sync.dma_start(outs[b][:, fsl], ob[:])
    nc.compile()
    return nc


def _host_precompute(depth, fl, cd):
    """Per-batch device inputs. Index math in float32, matching the jax
    reference op-for-op."""
    f32 = np.float32
    res = RES
    c = ((np.arange(res, dtype=f32) + f32(0.5)) / f32(res)) - f32(0.5)
    zc = f32(cd) - c                        # [k]
    kvalid = zc > 0
    with np.errstate(divide="ignore", invalid="ignore"):
        u = (f32(fl) * c)[:, None] / zc[None, :] + f32((IMG - 1) * 0.5)  # [i,k] == [j,k]
    ui = np.clip(np.round(u), 0, IMG - 1).astype(np.int64)
    mu = (u >= 0) & (u <= IMG - 1) & kvalid[None, :]

    if mu.any():
        cmin = int(ui[mu].min())
        cmax = int(ui[mu].max())
    else:
        cmin = cmax = 0
    if (cmax - cmin) >= WIN:
        raise NotImplementedError("projection span exceeds window")
    base = min(cmin, IMG - WIN)

    wd = depth[base:base + WIN, base:base + WIN].astype(f32)
    w = wd - f32(cd)
    w[wd <= 0] = POISON
    wpad = np.full((2 * P, WPAD), POISON, dtype=f32)
    wpad[:WIN, :WIN] = w
    w_hi = wpad.astype(np.float16)          # [256 r, 256 c]
    # winT tiles: wt[ct][c_within, r] = W'[r, 128*ct + c_within]
    wt = np.ascontiguousarray(w_hi.T).reshape(2, P, 2 * P)

    # index map per (k, i); invalid -> poison col/row 252
    uiw = np.where(mu, ui - base, WIN).astype(np.float16)      # [i, k]
    vi_rep = np.tile(uiw.T.reshape(1, NCHUNK * NF), (P, 1))

    zk = np.where(kvalid, c, POISON).astype(np.float16)
    zneg = np.repeat(zk, res)[None, :]

    return wt, vi_rep, zneg


def kernel(depth_t, fl, cam_dist):
    from concourse.bass_utils import run_bass_kernel_spmd

    depth_t = np.asarray(depth_t)
    fl = np.asarray(fl).reshape(N)
    cam_dist = np.asarray(cam_dist).reshape(N)

    if "nc" not in _nc_cache:
        _nc_cache["nc"] = _build_program()
    nc = _nc_cache["nc"]

    pcol = np.stack([np.arange(P, dtype=np.float32),
                     np.arange(P, dtype=np.float32) + P], axis=1)
    ones1 = np.ones((1, P), dtype=np.float16)

    in_maps = []
    for core in range(NCORES):
        m = {"pcol": pcol, "ones1": ones1}
        for b in range(BPC):
            g = core * BPC + b
            wt, vi_rep, zneg = _host_precompute(depth_t[g, 0], fl[g], cam_dist[g])
            m[f"wt{b}"] = wt
            m[f"vi{b}"] = vi_rep
            m[f"zneg{b}"] = zneg
        in_maps.append(m)

    globals()["_last_in_maps"] = in_maps
    r = run_bass_kernel_spmd(nc, in_maps, list(range(NCORES)))

    out = np.empty((N, 1, RES, RES, RES), dtype=np.float32)
    for core in range(NCORES):
        for b in range(BPC):
            g = core * BPC + b
            od = r.results[core][f"outdev{b}"].reshape(RES, RES, RES)  # [j,k,i]
            out[g, 0] = od.transpose(2, 0, 1)
    return out
